# revision 4
# baseline (speedup 1.0000x reference)
"""Trainium2 Bass kernel for nn_CKConv (SIREN kernel-net + causal conv1d).

Decomposition (8 cores, SPMD — identical program, per-core data):
  z[n,o] = sum_{ci, l<=n} W[o,ci,4095-l] * x[n-l,ci],  W[o,ci,m]=weights[m,32o+ci]
Tap l = 128u + 16c + 4g + dl  (c = core, u in [0,32), g,dl in [0,4)).
Each core computes SIREN weights for its 512 taps (j' = 16u+4g+dl, packed
[128,128]), writes them to DRAM, gathers them back as conv lhsT tiles
[(dl,ci),(g,o)], and runs 144 accumulating [K=128,M=128,N=512] matmuls against
a 4-shift x image XS.  psum row (g,o) of output tile T holds the contribution
to z[512T+dn+4g+16c, o]; the host sums the shifted partials.

Numerics: layer-1 sin args (up to ~3.3e5 rad) are computed with the reference's
exact fp32 rounding sequence and evaluated with a Cody-Waite range reduction
(6x7-bit pieces of 2pi; k*Pi products exact) + degree-11 odd polynomial, giving
|sin err| < 1e-6 and final rel err ~1e-5 vs the fp32 jax reference.
"""
import os
import sys
import numpy as np

if "/opt/trn_rl_repo" not in sys.path:
    sys.path.insert(0, "/opt/trn_rl_repo")

f32 = np.float32
OMEGA = 32.5
N, CIN, COUT, H = 4096, 32, 32, 32
NCORES = 8
PAD = 512
XSW = PAD + N + 4

INV2PI = 0.15915493667125702
MAGIC = 12582912.0  # 1.5 * 2^23
P6 = [6.3125, -0.029296875, -1.7881393432617188e-05, 6.332993507385254e-08,
      2.4374458007514477e-10, -6.608047442568932e-13]
Q2 = [6.2831878662109375, -2.559034328442067e-06]
CPOLY = [0.9999995827674866, -0.1666654497385025, 0.008332346566021442,
         -0.00019806942145805806, 2.697602667467436e-06, -2.0269567357900087e-08]

_CACHE = {}


def _build_program():
    import concourse.bacc as bacc
    import concourse.mybir as mybir
    import concourse.tile as tile

    dt = mybir.dt.float32
    AF = mybir.ActivationFunctionType
    OP = mybir.AluOpType

    nc = bacc.Bacc("TRN2", target_bir_lowering=False, debug=False,
                   num_devices=NCORES)

    # ---------------- DRAM I/O ----------------
    d_t1 = nc.dram_tensor("t1rep", [128, 128], dt, kind="ExternalInput")
    d_w1 = nc.dram_tensor("w1rep", [128, 1], dt, kind="ExternalInput")
    d_b1 = nc.dram_tensor("b1rep", [128, 1], dt, kind="ExternalInput")
    d_v2 = nc.dram_tensor("v2rep", [128, 32], dt, kind="ExternalInput")
    d_v2T = nc.dram_tensor("v2Trep", [128, 32], dt, kind="ExternalInput")
    d_og2 = nc.dram_tensor("omg2rep", [128, 1], dt, kind="ExternalInput")
    d_ob2 = nc.dram_tensor("omb2rep", [128, 1], dt, kind="ExternalInput")
    d_w3 = nc.dram_tensor("w3rep", [128, 1024], dt, kind="ExternalInput")
    d_b3 = nc.dram_tensor("b3rep", [128, 1024], dt, kind="ExternalInput")
    d_xT = nc.dram_tensor("xT", [32, 4096], dt, kind="ExternalInput")
    d_out = nc.dram_tensor("out", [8, 128, 512], dt, kind="ExternalOutput")

    with tile.TileContext(nc) as tc:
        with (
            tc.tile_pool(name="const", bufs=1) as cp,
            tc.tile_pool(name="sin", bufs=1) as sp,
            tc.tile_pool(name="wsb", bufs=2) as wp,
            tc.tile_pool(name="lhs", bufs=1) as lp,
            tc.tile_pool(name="osb", bufs=2) as op_,
            tc.tile_pool(name="ps2", bufs=1, space="PSUM") as pp2,
            tc.tile_pool(name="ps3", bufs=2, space="PSUM") as pp3,
            tc.tile_pool(name="psc", bufs=2, space="PSUM") as ppc,
            tc.tile_pool(name="wdram", bufs=1, space="DRAM") as dp,
        ):
            # ---------------- load constants ----------------
            def load(dram, shape, tag):
                t = cp.tile(shape, dt, tag=tag)
                nc.sync.dma_start(t[:], dram[:])
                return t

            t1 = load(d_t1, [128, 128], "t1")
            w1r = load(d_w1, [128, 1], "w1r")
            b1r = load(d_b1, [128, 1], "b1r")
            v2r = load(d_v2, [128, 32], "v2r")
            v2Tr = load(d_v2T, [128, 32], "v2Tr")
            og2 = load(d_og2, [128, 1], "og2")
            ob2 = load(d_ob2, [128, 1], "ob2")
            w3r = load(d_w3, [128, 1024], "w3r")
            b3r = load(d_b3, [128, 1024], "b3r")

            # XS: [128=(4ci+dl), XSW];  XS[(ci,dl), PAD+dl+j] = x[j,ci]
            xs = cp.tile([128, XSW], dt, tag="xs")
            nc.vector.memset(xs[:, 0:516], 0.0)
            xs_r = xs[:].rearrange("(ci dl) w -> dl ci w", dl=4)
            for dl in range(4):
                nc.sync.dma_start(
                    xs_r[dl, :, PAD + dl:PAD + dl + 4096], d_xT[:])

            # ---------------- sin helper ----------------
            def dev_sin(out, arg, pieces, nm):
                v = sp.tile([128, 128], dt, tag=f"{nm}v")
                k = sp.tile([128, 128], dt, tag=f"{nm}k")
                t_ = sp.tile([128, 128], dt, tag=f"{nm}t")
                r = sp.tile([128, 128], dt, tag=f"{nm}r")
                u = sp.tile([128, 128], dt, tag=f"{nm}u")
                a = sp.tile([128, 128], dt, tag=f"{nm}a")
                nc.vector.tensor_scalar(v[:], arg, INV2PI, MAGIC,
                                        op0=OP.mult, op1=OP.add)
                nc.vector.tensor_scalar_sub(k[:], v[:], MAGIC)
                first = True
                for p in pieces:
                    nc.vector.tensor_scalar_mul(t_[:], k[:], float(p))
                    nc.vector.tensor_sub(r[:], arg if first else r[:], t_[:])
                    first = False
                nc.vector.tensor_mul(u[:], r[:], r[:])
                nc.vector.tensor_scalar(a[:], u[:], CPOLY[5], CPOLY[4],
                                        op0=OP.mult, op1=OP.add)
                for ci in (3, 2, 1, 0):
                    nc.vector.tensor_mul(a[:], a[:], u[:])
                    nc.vector.tensor_scalar_add(a[:], a[:], CPOLY[ci])
                nc.vector.tensor_mul(out, a[:], r[:])

            # ---------------- layer 1 ----------------
            # arg1 = fl(fl(fl(t*w1)+b1)*OMEGA)  (reference's exact roundings)
            a1 = sp.tile([128, 128], dt, tag="a1")
            nc.vector.tensor_scalar(a1[:], t1[:], w1r[:, 0:1], b1r[:, 0:1],
                                    op0=OP.mult, op1=OP.add)
            nc.vector.tensor_scalar_mul(a1[:], a1[:], OMEGA)
            h1q = cp.tile([128, 128], dt, tag="h1q")
            dev_sin(h1q[:], a1[:], P6, "s1")

            # ---------------- layer 2 ----------------
            # psum2q[(q,h'),mm] = sum_h v2[h',h] * h1[h,128q+mm]
            ps2 = pp2.tile([128, 128], dt)
            for q in range(4):
                nc.tensor.matmul(ps2[32 * q:32 * q + 32, :],
                                 v2Tr[32 * q:32 * q + 32, :],
                                 h1q[32 * q:32 * q + 32, :],
                                 start=True, stop=True,
                                 tile_position=(32 * q, 32 * q))
            # scaleA = omg2 * 1/sqrt(sum v2^2)
            sq = sp.tile([128, 32], dt, tag="sq")
            n2 = sp.tile([128, 1], dt, tag="n2")
            sca = sp.tile([128, 1], dt, tag="sca")
            nc.vector.tensor_mul(sq[:], v2r[:], v2r[:])
            nc.vector.tensor_reduce(n2[:], sq[:], mybir.AxisListType.X, OP.add)
            nc.scalar.activation(n2[:], n2[:], AF.Sqrt)
            nc.vector.reciprocal(n2[:], n2[:])
            nc.vector.tensor_mul(sca[:], og2[:], n2[:])
            a2 = sp.tile([128, 128], dt, tag="a2")
            nc.vector.tensor_scalar(a2[:], ps2[:], sca[:, 0:1], ob2[:, 0:1],
                                    op0=OP.mult, op1=OP.add)
            h2q = cp.tile([128, 128], dt, tag="h2q")
            dev_sin(h2q[:], a2[:], Q2, "s2")

            # ---------------- layer 3 + gather + conv ----------------
            # Wnat[128Jb+mm, f] = sum_h h2q[32Jb+h, mm] * w3rep[32Jb+h, f] + b3
            lhsTb = []
            for Jb in range(4):
                ps3 = pp3.tile([128, 1024], dt)
                for fb in range(2):
                    nc.tensor.matmul(ps3[:, 512 * fb:512 * fb + 512],
                                     h2q[32 * Jb:32 * Jb + 32, :],
                                     w3r[32 * Jb:32 * Jb + 32,
                                         512 * fb:512 * fb + 512],
                                     start=True, stop=True,
                                     tile_position=(32 * Jb, 0))
                wsb = wp.tile([128, 1024], dt, tag="wsb")
                nc.vector.tensor_add(wsb[:], ps3[:], b3r[:])
                wd = dp.tile([128, 1024], dt, tag=f"wd{Jb}")
                nc.sync.dma_start(wd[:], wsb[:])
                # gather: lhsTb[Jb][(4ci+dl), 128us+32g+o] = Wnat[16us+4g+dl, 32ci+o]
                lb = lp.tile([128, 1024], dt, tag=f"lb{Jb}")
                src = wd[:].rearrange("(usg dl) (ci o) -> dl ci usg o",
                                      dl=4, o=32)
                dst = lb[:].rearrange("(ci dl) f -> dl ci f", dl=4)
                for dl in range(4):
                    nc.sync.dma_start(dst[dl], src[dl])
                lhsTb.append(lb)

            # conv: for output tile T accumulate chunks u = 0..4(T+1)-1
            for T in range(8):
                nu = 4 * (T + 1)
                psc = ppc.tile([128, 512], dt)
                for u in range(nu):
                    Jb, us = u // 8, u % 8
                    off = PAD + 512 * T - 128 * u
                    nc.tensor.matmul(psc[:],
                                     lhsTb[Jb][:, 128 * us:128 * us + 128],
                                     xs[:, off:off + 512],
                                     start=(u == 0), stop=(u == nu - 1))
                osb = op_.tile([128, 512], dt, tag="osb")
                nc.vector.tensor_copy(osb[:], psc[:])
                nc.sync.dma_start(d_out[T], osb[:])

    nc.finalize()
    return nc


def _host_prep(inputs):
    """Per-core input maps.  Only t1rep differs across cores."""
    import jax
    import jax.numpy as jnp
    cpu = jax.devices("cpu")[0]
    with jax.default_device(cpu):
        t_new = np.asarray(jnp.linspace(-1.0, 2.0 * (N / 1.0) - 1.0, N))
    t_new = t_new.astype(f32)

    x = np.asarray(inputs["x"], dtype=f32)
    v1 = np.asarray(inputs["v1"], dtype=f32)
    g1 = np.asarray(inputs["g1"], dtype=f32)
    b1 = np.asarray(inputs["b1"], dtype=f32)
    v2 = np.asarray(inputs["v2"], dtype=f32)
    g2 = np.asarray(inputs["g2"], dtype=f32)
    b2 = np.asarray(inputs["b2"], dtype=f32)
    w3 = np.asarray(inputs["w3"], dtype=f32)
    b3 = np.asarray(inputs["b3"], dtype=f32)

    w1 = (g1[:, None] * v1 / np.linalg.norm(v1, axis=1, keepdims=True)
          ).astype(f32)[:, 0]

    common = {
        "w1rep": np.ascontiguousarray(np.tile(w1, 4)[:, None]),
        "b1rep": np.ascontiguousarray(np.tile(b1, 4)[:, None]),
        "v2rep": np.ascontiguousarray(np.tile(v2, (4, 1))),
        "v2Trep": np.ascontiguousarray(np.tile(v2.T, (4, 1))),
        "omg2rep": np.ascontiguousarray(
            f32(OMEGA * np.tile(g2, 4))[:, None]),
        "omb2rep": np.ascontiguousarray(
            f32(OMEGA * np.tile(b2, 4))[:, None]),
        "w3rep": np.ascontiguousarray(np.tile(
            np.transpose(w3.reshape(COUT, CIN, H), (2, 1, 0)
                         ).reshape(H, CIN * COUT), (4, 1))),
        "b3rep": np.ascontiguousarray(np.tile(
            b3.reshape(COUT, CIN).T.reshape(-1), (128, 1))),
        "xT": np.ascontiguousarray(x.T),
    }
    jp = np.arange(512)
    in_maps = []
    for c in range(NCORES):
        m_of = 4095 - 16 * c - 128 * (jp // 16) - (jp % 16)
        tq = t_new[m_of].reshape(4, 128)
        t1 = np.repeat(tq[:, None, :], 32, axis=1).reshape(128, 128)
        im = dict(common)
        im["t1rep"] = np.ascontiguousarray(t1)
        in_maps.append(im)
    return in_maps


def _host_combine(outs):
    Z = np.zeros((N + 256, COUT), dtype=np.float64)
    for c in range(NCORES):
        oc = np.asarray(outs[c]).reshape(8, 4, 32, 512)  # [T, g, o, dn]
        for g in range(4):
            seq = oc[:, g].transpose(0, 2, 1).reshape(N, COUT)
            s = 16 * c + 4 * g
            Z[s:s + N] += seq
    return Z[:N].astype(f32)


def kernel(**inputs):
    from concourse import bass_utils

    t = np.asarray(inputs["t"])
    t_min = int(t.min())
    idx = t - t_min
    assert int(t.max()) - t_min + 1 == N, "kernel hardcodes N=4096"

    # scatter observations onto the regular grid (identity when t is arange)
    x_in = np.asarray(inputs["x"], dtype=f32)
    x_new = np.zeros((N, CIN), dtype=f32)
    x_new[idx] = x_in
    ins = dict(inputs)
    ins["x"] = x_new

    if "prog" not in _CACHE:
        _CACHE["prog"] = _build_program()
    nc = _CACHE["prog"]

    in_maps = _host_prep(ins)
    res = bass_utils.run_bass_kernel_spmd(
        nc, in_maps, core_ids=list(range(NCORES)))
    outs = [res.results[c]["out"] for c in range(NCORES)]
    z = _host_combine(outs)
    return z[idx]


if __name__ == "__main__":
    import jax
    cpu = jax.devices("cpu")[0]
    with jax.default_device(cpu):
        sys.path.insert(0, os.path.dirname(os.path.abspath(__file__)))
        import reference as R
        inputs = {k: np.asarray(v) for k, v in R.setup_inputs().items()}
        import jax.numpy as jnp
        z0 = np.asarray(R.reference(**{k: jnp.asarray(v)
                                       for k, v in inputs.items()}))
    z = kernel(**inputs)
    rel = np.linalg.norm(z - z0) / np.linalg.norm(z0)
    print("Relative error:", rel)


# revision 7
# speedup vs baseline: 2.6437x; 2.6437x over previous
"""Trainium2 Bass kernel for nn_CKConv (SIREN kernel-net + causal conv1d).

Decomposition (8 cores, SPMD — identical program, per-core data):
  z[n,o] = sum_{ci, l<=n} W[o,ci,4095-l] * x[n-l,ci],  W[o,ci,m]=weights[m,32o+ci]
Tap l = 128u + 16c + 4g + dl  (c = core, u in [0,32), g,dl in [0,4)).
Each core computes SIREN weights for its 512 taps (j' = 16u+4g+dl, packed
[128,128]), writes them to DRAM, gathers them back as conv lhsT tiles
[(dl,ci),(g,o)], and runs 144 accumulating [K=128,M=128,N=512] matmuls against
a 4-shift x image XS.  psum row (g,o) of output tile T holds the contribution
to z[512T+dn+4g+16c, o]; the host sums the shifted partials.

Numerics: layer-1 sin args (up to ~3.3e5 rad) are computed with the reference's
exact fp32 rounding sequence and evaluated with a Cody-Waite range reduction
(6x7-bit pieces of 2pi; k*Pi products exact) + degree-11 odd polynomial, giving
|sin err| < 1e-6 and final rel err ~1e-5 vs the fp32 jax reference.
"""
import os
import sys
import numpy as np

if "/opt/trn_rl_repo" not in sys.path:
    sys.path.insert(0, "/opt/trn_rl_repo")

f32 = np.float32
OMEGA = 32.5
N, CIN, COUT, H = 4096, 32, 32, 32
NCORES = 8
PAD = 512
XSW = PAD + N + 4

INV2PI = 0.15915493667125702
MAGIC = 12582912.0  # 1.5 * 2^23
P6 = [6.3125, -0.029296875, -1.7881393432617188e-05, 6.332993507385254e-08,
      2.4374458007514477e-10, -6.608047442568932e-13]
Q2 = [6.2831878662109375, -2.559034328442067e-06]
CPOLY = [0.9999995827674866, -0.1666654497385025, 0.008332346566021442,
         -0.00019806942145805806, 2.697602667467436e-06, -2.0269567357900087e-08]

_CACHE = {}


def _build_program():
    import concourse.bacc as bacc
    import concourse.mybir as mybir
    import concourse.tile as tile

    dt = mybir.dt.float32
    AF = mybir.ActivationFunctionType
    OP = mybir.AluOpType

    nc = bacc.Bacc("TRN2", target_bir_lowering=False, debug=False,
                   num_devices=NCORES)

    # ---------------- DRAM I/O ----------------
    d_t1 = nc.dram_tensor("t1rep", [128, 128], dt, kind="ExternalInput")
    d_w1 = nc.dram_tensor("w1rep", [128, 1], dt, kind="ExternalInput")
    d_b1 = nc.dram_tensor("b1rep", [128, 1], dt, kind="ExternalInput")
    d_v2 = nc.dram_tensor("v2rep", [128, 32], dt, kind="ExternalInput")
    d_v2T = nc.dram_tensor("v2Trep", [128, 32], dt, kind="ExternalInput")
    d_og2 = nc.dram_tensor("omg2rep", [128, 1], dt, kind="ExternalInput")
    d_ob2 = nc.dram_tensor("omb2rep", [128, 1], dt, kind="ExternalInput")
    d_w3 = nc.dram_tensor("w3rep", [128, 1024], mybir.dt.float16,
                          kind="ExternalInput")
    d_b3 = nc.dram_tensor("b3rep", [128, 1024], dt, kind="ExternalInput")
    d_xT = nc.dram_tensor("xT", [32, 4096], mybir.dt.float16,
                          kind="ExternalInput")
    d_out = nc.dram_tensor("out", [8, 128, 512], dt, kind="ExternalOutput")

    with tile.TileContext(nc) as tc:
        with (
            tc.tile_pool(name="const", bufs=1) as cp,
            tc.tile_pool(name="sin", bufs=1) as sp,
            tc.tile_pool(name="wsb", bufs=2) as wp,
            tc.tile_pool(name="lhs", bufs=1) as lp,
            tc.tile_pool(name="osb", bufs=2) as op_,
            tc.tile_pool(name="ps2", bufs=1, space="PSUM") as pp2,
            tc.tile_pool(name="ps3", bufs=2, space="PSUM") as pp3,
            tc.tile_pool(name="psc", bufs=2, space="PSUM") as ppc,
            tc.tile_pool(name="wdram", bufs=1, space="DRAM") as dp,
        ):
            # ---------------- load constants ----------------
            def load(dram, shape, tag):
                t = cp.tile(shape, dt, tag=tag)
                nc.sync.dma_start(t[:], dram[:])
                return t

            t1 = load(d_t1, [128, 128], "t1")
            w1r = load(d_w1, [128, 1], "w1r")
            b1r = load(d_b1, [128, 1], "b1r")
            v2r = load(d_v2, [128, 32], "v2r")
            v2Tr = load(d_v2T, [128, 32], "v2Tr")
            og2 = load(d_og2, [128, 1], "og2")
            ob2 = load(d_ob2, [128, 1], "ob2")
            w3r = cp.tile([128, 1024], mybir.dt.float16, tag="w3r")
            nc.sync.dma_start(w3r[:], d_w3[:])
            b3r = load(d_b3, [128, 1024], "b3r")

            # XS: [128=(4ci+dl), XSW];  XS[(ci,dl), PAD+dl+j] = x[j,ci]
            xs = cp.tile([128, XSW], mybir.dt.float16, tag="xs")
            nc.vector.memset(xs[:, 0:516], 0.0)
            xs_r = xs[:].rearrange("(ci dl) w -> dl ci w", dl=4)
            for dl in range(4):
                nc.sync.dma_start(
                    xs_r[dl, :, PAD + dl:PAD + dl + 4096], d_xT[:])

            # ---------------- sin helper ----------------
            def dev_sin(out, arg, pieces, nm):
                v = sp.tile([128, 128], dt, tag=f"{nm}v")
                k = sp.tile([128, 128], dt, tag=f"{nm}k")
                t_ = sp.tile([128, 128], dt, tag=f"{nm}t")
                r = sp.tile([128, 128], dt, tag=f"{nm}r")
                u = sp.tile([128, 128], dt, tag=f"{nm}u")
                a = sp.tile([128, 128], dt, tag=f"{nm}a")
                nc.vector.tensor_scalar(v[:], arg, INV2PI, MAGIC,
                                        op0=OP.mult, op1=OP.add)
                nc.vector.tensor_scalar_sub(k[:], v[:], MAGIC)
                first = True
                for p in pieces:
                    nc.vector.tensor_scalar_mul(t_[:], k[:], float(p))
                    nc.vector.tensor_sub(r[:], arg if first else r[:], t_[:])
                    first = False
                nc.vector.tensor_mul(u[:], r[:], r[:])
                nc.vector.tensor_scalar(a[:], u[:], CPOLY[5], CPOLY[4],
                                        op0=OP.mult, op1=OP.add)
                for ci in (3, 2, 1, 0):
                    nc.vector.tensor_mul(a[:], a[:], u[:])
                    nc.vector.tensor_scalar_add(a[:], a[:], CPOLY[ci])
                nc.vector.tensor_mul(out, a[:], r[:])

            # ---------------- layer 1 ----------------
            # arg1 = fl(fl(fl(t*w1)+b1)*OMEGA)  (reference's exact roundings)
            a1 = sp.tile([128, 128], dt, tag="a1")
            nc.vector.tensor_scalar(a1[:], t1[:], w1r[:, 0:1], b1r[:, 0:1],
                                    op0=OP.mult, op1=OP.add)
            nc.vector.tensor_scalar_mul(a1[:], a1[:], OMEGA)
            h1q = cp.tile([128, 128], dt, tag="h1q")
            dev_sin(h1q[:], a1[:], P6, "s1")

            # ---------------- layer 2 ----------------
            # psum2q[(q,h'),mm] = sum_h v2[h',h] * h1[h,128q+mm]
            ps2 = pp2.tile([128, 128], dt)
            for q in range(4):
                nc.tensor.matmul(ps2[32 * q:32 * q + 32, :],
                                 v2Tr[32 * q:32 * q + 32, :],
                                 h1q[32 * q:32 * q + 32, :],
                                 start=True, stop=True,
                                 tile_position=(32 * q, 32 * q))
            # scaleA = omg2 * 1/sqrt(sum v2^2)
            sq = sp.tile([128, 32], dt, tag="sq")
            n2 = sp.tile([128, 1], dt, tag="n2")
            sca = sp.tile([128, 1], dt, tag="sca")
            nc.vector.tensor_mul(sq[:], v2r[:], v2r[:])
            nc.vector.tensor_reduce(n2[:], sq[:], mybir.AxisListType.X, OP.add)
            nc.scalar.activation(n2[:], n2[:], AF.Sqrt)
            nc.vector.reciprocal(n2[:], n2[:])
            nc.vector.tensor_mul(sca[:], og2[:], n2[:])
            a2 = sp.tile([128, 128], dt, tag="a2")
            nc.vector.tensor_scalar(a2[:], ps2[:], sca[:, 0:1], ob2[:, 0:1],
                                    op0=OP.mult, op1=OP.add)
            h2q = cp.tile([128, 128], mybir.dt.float16, tag="h2q")
            dev_sin(h2q[:], a2[:], Q2, "s2")

            # ---------------- layer 3 + gather + conv ----------------
            # Wnat[128Jb+mm, f] = sum_h h2q[32Jb+h, mm] * w3rep[32Jb+h, f] + b3
            lhsTb = []
            for Jb in range(4):
                ps3 = pp3.tile([128, 1024], dt)
                for fb in range(2):
                    nc.tensor.matmul(ps3[:, 512 * fb:512 * fb + 512],
                                     h2q[32 * Jb:32 * Jb + 32, :],
                                     w3r[32 * Jb:32 * Jb + 32,
                                         512 * fb:512 * fb + 512],
                                     start=True, stop=True,
                                     tile_position=(32 * Jb, 0))
                wsb = wp.tile([128, 1024], mybir.dt.float16, tag="wsb")
                nc.vector.tensor_add(wsb[:], ps3[:], b3r[:])
                wd = dp.tile([128, 1024], mybir.dt.float16, tag=f"wd{Jb}")
                nc.sync.dma_start(wd[:], wsb[:])
                # gather: lhsTb[Jb][(4ci+dl), 128us+32g+o] = Wnat[16us+4g+dl, 32ci+o]
                lb = lp.tile([128, 1024], mybir.dt.float16, tag=f"lb{Jb}")
                src = wd[:].rearrange("(usg dl) (ci o) -> dl ci usg o",
                                      dl=4, o=32)
                dst = lb[:].rearrange("(ci dl) f -> dl ci f", dl=4)
                for dl in range(4):
                    nc.sync.dma_start(dst[dl], src[dl])
                lhsTb.append(lb)

            # conv: for output tile T accumulate chunks u = 0..4(T+1)-1
            for T in range(8):
                nu = 4 * (T + 1)
                psc = ppc.tile([128, 512], dt)
                for u in range(nu):
                    Jb, us = u // 8, u % 8
                    off = PAD + 512 * T - 128 * u
                    nc.tensor.matmul(psc[:],
                                     lhsTb[Jb][:, 128 * us:128 * us + 128],
                                     xs[:, off:off + 512],
                                     start=(u == 0), stop=(u == nu - 1))
                osb = op_.tile([128, 512], dt, tag="osb")
                nc.vector.tensor_copy(osb[:], psc[:])
                nc.sync.dma_start(d_out[T], osb[:])

    nc.finalize()
    return nc


def _host_prep(inputs):
    """Per-core input maps.  Only t1rep differs across cores."""
    import jax
    import jax.numpy as jnp
    cpu = jax.devices("cpu")[0]
    with jax.default_device(cpu):
        t_new = np.asarray(jnp.linspace(-1.0, 2.0 * (N / 1.0) - 1.0, N))
    t_new = t_new.astype(f32)

    x = np.asarray(inputs["x"], dtype=f32)
    v1 = np.asarray(inputs["v1"], dtype=f32)
    g1 = np.asarray(inputs["g1"], dtype=f32)
    b1 = np.asarray(inputs["b1"], dtype=f32)
    v2 = np.asarray(inputs["v2"], dtype=f32)
    g2 = np.asarray(inputs["g2"], dtype=f32)
    b2 = np.asarray(inputs["b2"], dtype=f32)
    w3 = np.asarray(inputs["w3"], dtype=f32)
    b3 = np.asarray(inputs["b3"], dtype=f32)

    w1 = (g1[:, None] * v1 / np.linalg.norm(v1, axis=1, keepdims=True)
          ).astype(f32)[:, 0]

    common = {
        "w1rep": np.ascontiguousarray(np.tile(w1, 4)[:, None]),
        "b1rep": np.ascontiguousarray(np.tile(b1, 4)[:, None]),
        "v2rep": np.ascontiguousarray(np.tile(v2, (4, 1))),
        "v2Trep": np.ascontiguousarray(np.tile(v2.T, (4, 1))),
        "omg2rep": np.ascontiguousarray(
            f32(OMEGA * np.tile(g2, 4))[:, None]),
        "omb2rep": np.ascontiguousarray(
            f32(OMEGA * np.tile(b2, 4))[:, None]),
        "w3rep": np.ascontiguousarray(np.tile(
            np.transpose(w3.reshape(COUT, CIN, H), (2, 1, 0)
                         ).reshape(H, CIN * COUT), (4, 1))).astype(np.float16),
        "b3rep": np.ascontiguousarray(np.tile(
            b3.reshape(COUT, CIN).T.reshape(-1), (128, 1))),
        "xT": np.ascontiguousarray(x.T).astype(np.float16),
    }
    jp = np.arange(512)
    in_maps = []
    for c in range(NCORES):
        m_of = 4095 - 16 * c - 128 * (jp // 16) - (jp % 16)
        tq = t_new[m_of].reshape(4, 128)
        t1 = np.repeat(tq[:, None, :], 32, axis=1).reshape(128, 128)
        im = dict(common)
        im["t1rep"] = np.ascontiguousarray(t1)
        in_maps.append(im)
    return in_maps


def _host_combine(outs):
    Z = np.zeros((N + 256, COUT), dtype=np.float64)
    for c in range(NCORES):
        oc = np.asarray(outs[c]).reshape(8, 4, 32, 512)  # [T, g, o, dn]
        for g in range(4):
            seq = oc[:, g].transpose(0, 2, 1).reshape(N, COUT)
            s = 16 * c + 4 * g
            Z[s:s + N] += seq
    return Z[:N].astype(f32)


def kernel(**inputs):
    from concourse import bass_utils

    t = np.asarray(inputs["t"])
    t_min = int(t.min())
    idx = t - t_min
    assert int(t.max()) - t_min + 1 == N, "kernel hardcodes N=4096"

    # scatter observations onto the regular grid (identity when t is arange)
    x_in = np.asarray(inputs["x"], dtype=f32)
    x_new = np.zeros((N, CIN), dtype=f32)
    x_new[idx] = x_in
    ins = dict(inputs)
    ins["x"] = x_new

    if "prog" not in _CACHE:
        _CACHE["prog"] = _build_program()
    nc = _CACHE["prog"]

    in_maps = _host_prep(ins)
    res = bass_utils.run_bass_kernel_spmd(
        nc, in_maps, core_ids=list(range(NCORES)))
    outs = [res.results[c]["out"] for c in range(NCORES)]
    z = _host_combine(outs)
    return z[idx]


if __name__ == "__main__":
    import jax
    cpu = jax.devices("cpu")[0]
    with jax.default_device(cpu):
        sys.path.insert(0, os.path.dirname(os.path.abspath(__file__)))
        import reference as R
        inputs = {k: np.asarray(v) for k, v in R.setup_inputs().items()}
        import jax.numpy as jnp
        z0 = np.asarray(R.reference(**{k: jnp.asarray(v)
                                       for k, v in inputs.items()}))
    z = kernel(**inputs)
    rel = np.linalg.norm(z - z0) / np.linalg.norm(z0)
    print("Relative error:", rel)


# revision 9
# speedup vs baseline: 2.7223x; 1.0297x over previous
"""Trainium2 Bass kernel for nn_CKConv (SIREN kernel-net + causal conv1d).

Decomposition (8 cores, SPMD — identical program, per-core data):
  z[n,o] = sum_{ci, l<=n} W[o,ci,4095-l] * x[n-l,ci],  W[o,ci,m]=weights[m,32o+ci]
Tap l = 128u + 16c + 4g + dl  (c = core, u in [0,32), g,dl in [0,4)).
Each core computes SIREN weights for its 512 taps (j' = 16u+4g+dl, packed
[128,128]), writes them to DRAM, gathers them back as conv lhsT tiles
[(dl,ci),(g,o)], and runs 144 accumulating [K=128,M=128,N=512] matmuls against
a 4-shift x image XS.  psum row (g,o) of output tile T holds the contribution
to z[512T+dn+4g+16c, o]; the host sums the shifted partials.

Numerics: layer-1 sin args (up to ~3.3e5 rad) are computed with the reference's
exact fp32 rounding sequence and evaluated with a Cody-Waite range reduction
(6x7-bit pieces of 2pi; k*Pi products exact) + degree-11 odd polynomial, giving
|sin err| < 1e-6 and final rel err ~1e-5 vs the fp32 jax reference.
"""
import os
import sys
import numpy as np

if "/opt/trn_rl_repo" not in sys.path:
    sys.path.insert(0, "/opt/trn_rl_repo")

f32 = np.float32
OMEGA = 32.5
N, CIN, COUT, H = 4096, 32, 32, 32
NCORES = 8
PAD = 512
XSW = PAD + N + 4

INV2PI = 0.15915493667125702
MAGIC = 12582912.0  # 1.5 * 2^23
P6 = [6.3125, -0.029296875, -1.7881393432617188e-05, 6.332993507385254e-08,
      2.4374458007514477e-10, -6.608047442568932e-13]
Q2 = [6.2831878662109375, -2.559034328442067e-06]
CPOLY = [0.9999995827674866, -0.1666654497385025, 0.008332346566021442,
         -0.00019806942145805806, 2.697602667467436e-06, -2.0269567357900087e-08]

_CACHE = {}


def _build_program():
    import concourse.bacc as bacc
    import concourse.mybir as mybir
    import concourse.tile as tile

    dt = mybir.dt.float32
    AF = mybir.ActivationFunctionType
    OP = mybir.AluOpType

    nc = bacc.Bacc("TRN2", target_bir_lowering=False, debug=False,
                   num_devices=NCORES)

    # ---------------- DRAM I/O ----------------
    d_t1 = nc.dram_tensor("t1rep", [128, 128], dt, kind="ExternalInput")
    d_w1 = nc.dram_tensor("w1rep", [128, 1], dt, kind="ExternalInput")
    d_b1 = nc.dram_tensor("b1rep", [128, 1], dt, kind="ExternalInput")
    d_v2 = nc.dram_tensor("v2rep", [128, 32], dt, kind="ExternalInput")
    d_v2T = nc.dram_tensor("v2Trep", [128, 32], dt, kind="ExternalInput")
    d_og2 = nc.dram_tensor("omg2rep", [128, 1], dt, kind="ExternalInput")
    d_ob2 = nc.dram_tensor("omb2rep", [128, 1], dt, kind="ExternalInput")
    d_w3 = nc.dram_tensor("w3rep", [128, 1024], mybir.dt.float16,
                          kind="ExternalInput")
    d_b3 = nc.dram_tensor("b3rep", [128, 1024], dt, kind="ExternalInput")
    d_xT = nc.dram_tensor("xT", [32, 4096], mybir.dt.float16,
                          kind="ExternalInput")
    d_out = nc.dram_tensor("out", [8, 128, 512], dt, kind="ExternalOutput")

    with tile.TileContext(nc) as tc:
        with (
            tc.tile_pool(name="const", bufs=1) as cp,
            tc.tile_pool(name="sin", bufs=1) as sp,
            tc.tile_pool(name="wsb", bufs=2) as wp,
            tc.tile_pool(name="lhs", bufs=1) as lp,
            tc.tile_pool(name="osb", bufs=2) as op_,
            tc.tile_pool(name="ps2", bufs=1, space="PSUM") as pp2,
            tc.tile_pool(name="ps3", bufs=1, space="PSUM") as pp3,
            tc.tile_pool(name="psc", bufs=1, space="PSUM") as ppc,
            tc.tile_pool(name="wdram", bufs=1, space="DRAM") as dp,
        ):
            # ---------------- load constants ----------------
            def load(dram, shape, tag):
                t = cp.tile(shape, dt, tag=tag)
                nc.sync.dma_start(t[:], dram[:])
                return t

            t1 = load(d_t1, [128, 128], "t1")
            w1r = load(d_w1, [128, 1], "w1r")
            b1r = load(d_b1, [128, 1], "b1r")
            v2r = load(d_v2, [128, 32], "v2r")
            v2Tr = load(d_v2T, [128, 32], "v2Tr")
            og2 = load(d_og2, [128, 1], "og2")
            ob2 = load(d_ob2, [128, 1], "ob2")
            w3r = cp.tile([128, 1024], mybir.dt.float16, tag="w3r")
            nc.sync.dma_start(w3r[:], d_w3[:])
            b3r = load(d_b3, [128, 1024], "b3r")

            # XS: [128=(4ci+dl), XSW];  XS[(ci,dl), PAD+dl+j] = x[j,ci]
            xs = cp.tile([128, XSW], mybir.dt.float16, tag="xs")
            nc.vector.memset(xs[:, 0:516], 0.0)
            xs_r = xs[:].rearrange("(ci dl) w -> dl ci w", dl=4)
            for dl in range(4):
                nc.sync.dma_start(
                    xs_r[dl, :, PAD + dl:PAD + dl + 4096], d_xT[:])

            # ---------------- sin helper ----------------
            def dev_sin(out, arg, pieces, nm):
                v = sp.tile([128, 128], dt, tag=f"{nm}v")
                k = sp.tile([128, 128], dt, tag=f"{nm}k")
                t_ = sp.tile([128, 128], dt, tag=f"{nm}t")
                r = sp.tile([128, 128], dt, tag=f"{nm}r")
                u = sp.tile([128, 128], dt, tag=f"{nm}u")
                a = sp.tile([128, 128], dt, tag=f"{nm}a")
                nc.vector.tensor_scalar(v[:], arg, INV2PI, MAGIC,
                                        op0=OP.mult, op1=OP.add)
                nc.vector.tensor_scalar_sub(k[:], v[:], MAGIC)
                first = True
                for p in pieces:
                    nc.vector.tensor_scalar_mul(t_[:], k[:], float(p))
                    nc.vector.tensor_sub(r[:], arg if first else r[:], t_[:])
                    first = False
                nc.vector.tensor_mul(u[:], r[:], r[:])
                nc.vector.tensor_scalar(a[:], u[:], CPOLY[5], CPOLY[4],
                                        op0=OP.mult, op1=OP.add)
                for ci in (3, 2, 1, 0):
                    nc.vector.tensor_mul(a[:], a[:], u[:])
                    nc.vector.tensor_scalar_add(a[:], a[:], CPOLY[ci])
                nc.vector.tensor_mul(out, a[:], r[:])

            # ---------------- layer 1 ----------------
            # arg1 = fl(fl(fl(t*w1)+b1)*OMEGA)  (reference's exact roundings)
            a1 = sp.tile([128, 128], dt, tag="a1")
            nc.vector.tensor_scalar(a1[:], t1[:], w1r[:, 0:1], b1r[:, 0:1],
                                    op0=OP.mult, op1=OP.add)
            nc.vector.tensor_scalar_mul(a1[:], a1[:], OMEGA)
            h1q = cp.tile([128, 128], dt, tag="h1q")
            dev_sin(h1q[:], a1[:], P6, "s1")

            # ---------------- layer 2 ----------------
            # psum2q[(q,h'),mm] = sum_h v2[h',h] * h1[h,128q+mm]
            ps2 = pp2.tile([128, 128], dt)
            for q in range(4):
                nc.tensor.matmul(ps2[32 * q:32 * q + 32, :],
                                 v2Tr[32 * q:32 * q + 32, :],
                                 h1q[32 * q:32 * q + 32, :],
                                 start=True, stop=True,
                                 tile_position=(32 * q, 32 * q))
            # scaleA = omg2 * 1/sqrt(sum v2^2)
            sq = sp.tile([128, 32], dt, tag="sq")
            n2 = sp.tile([128, 1], dt, tag="n2")
            sca = sp.tile([128, 1], dt, tag="sca")
            nc.vector.tensor_mul(sq[:], v2r[:], v2r[:])
            nc.vector.tensor_reduce(n2[:], sq[:], mybir.AxisListType.X, OP.add)
            nc.scalar.activation(n2[:], n2[:], AF.Sqrt)
            nc.vector.reciprocal(n2[:], n2[:])
            nc.vector.tensor_mul(sca[:], og2[:], n2[:])
            a2 = sp.tile([128, 128], dt, tag="a2")
            nc.vector.tensor_scalar(a2[:], ps2[:], sca[:, 0:1], ob2[:, 0:1],
                                    op0=OP.mult, op1=OP.add)
            h2q = cp.tile([128, 128], mybir.dt.float16, tag="h2q")
            dev_sin(h2q[:], a2[:], Q2, "s2")

            # ---------------- layer 3 + gather + conv ----------------
            # Wnat[128Jb+mm, f] = sum_h h2q[32Jb+h, mm] * w3rep[32Jb+h, f] + b3
            lhsTb = []
            for Jb in range(4):
                ps3 = pp3.tile([128, 1024], dt)
                for fb in range(2):
                    nc.tensor.matmul(ps3[:, 512 * fb:512 * fb + 512],
                                     h2q[32 * Jb:32 * Jb + 32, :],
                                     w3r[32 * Jb:32 * Jb + 32,
                                         512 * fb:512 * fb + 512],
                                     start=True, stop=True,
                                     tile_position=(32 * Jb, 0))
                wsb = wp.tile([128, 1024], mybir.dt.float16, tag="wsb")
                nc.vector.tensor_add(wsb[:], ps3[:], b3r[:])
                wd = dp.tile([128, 1024], mybir.dt.float16, tag=f"wd{Jb}")
                nc.sync.dma_start(wd[:], wsb[:])
                # gather: lhsTb[Jb][(4ci+dl), 128us+32g+o] = Wnat[16us+4g+dl, 32ci+o]
                lb = lp.tile([128, 1024], mybir.dt.float16, tag=f"lb{Jb}")
                src = wd[:].rearrange("(usg dl) (ci o) -> dl ci usg o",
                                      dl=4, o=32)
                dst = lb[:].rearrange("(ci dl) f -> dl ci f", dl=4)
                for dl in range(4):
                    nc.sync.dma_start(dst[dl], src[dl])
                lhsTb.append(lb)

            # conv: u-outer over T-halves so each stationary lhsT_u serves
            # up to 4 output tiles per LDWEIGHTS (PE.SEQ is the bottleneck)
            for half in (range(0, 4), range(4, 8)):
                pscs = {T: ppc.tile([128, 512], dt, tag=f"c{T % 4}",
                                    name=f"psc{T}")
                        for T in half}
                numax = 4 * (half[-1] + 1)
                for u in range(numax):
                    Jb, us = u // 8, u % 8
                    lt = lhsTb[Jb][:, 128 * us:128 * us + 128]
                    for T in half:
                        nu = 4 * (T + 1)
                        if u >= nu:
                            continue
                        off = PAD + 512 * T - 128 * u
                        nc.tensor.matmul(pscs[T][:], lt,
                                         xs[:, off:off + 512],
                                         start=(u == 0), stop=(u == nu - 1))
                        if u == nu - 1:
                            osb = op_.tile([128, 512], dt, tag="osb")
                            nc.vector.tensor_copy(osb[:], pscs[T][:])
                            nc.sync.dma_start(d_out[T], osb[:])

    nc.finalize()
    return nc


def _host_prep(inputs):
    """Per-core input maps.  Only t1rep differs across cores."""
    import jax
    import jax.numpy as jnp
    cpu = jax.devices("cpu")[0]
    with jax.default_device(cpu):
        t_new = np.asarray(jnp.linspace(-1.0, 2.0 * (N / 1.0) - 1.0, N))
    t_new = t_new.astype(f32)

    x = np.asarray(inputs["x"], dtype=f32)
    v1 = np.asarray(inputs["v1"], dtype=f32)
    g1 = np.asarray(inputs["g1"], dtype=f32)
    b1 = np.asarray(inputs["b1"], dtype=f32)
    v2 = np.asarray(inputs["v2"], dtype=f32)
    g2 = np.asarray(inputs["g2"], dtype=f32)
    b2 = np.asarray(inputs["b2"], dtype=f32)
    w3 = np.asarray(inputs["w3"], dtype=f32)
    b3 = np.asarray(inputs["b3"], dtype=f32)

    w1 = (g1[:, None] * v1 / np.linalg.norm(v1, axis=1, keepdims=True)
          ).astype(f32)[:, 0]

    common = {
        "w1rep": np.ascontiguousarray(np.tile(w1, 4)[:, None]),
        "b1rep": np.ascontiguousarray(np.tile(b1, 4)[:, None]),
        "v2rep": np.ascontiguousarray(np.tile(v2, (4, 1))),
        "v2Trep": np.ascontiguousarray(np.tile(v2.T, (4, 1))),
        "omg2rep": np.ascontiguousarray(
            f32(OMEGA * np.tile(g2, 4))[:, None]),
        "omb2rep": np.ascontiguousarray(
            f32(OMEGA * np.tile(b2, 4))[:, None]),
        "w3rep": np.ascontiguousarray(np.tile(
            np.transpose(w3.reshape(COUT, CIN, H), (2, 1, 0)
                         ).reshape(H, CIN * COUT), (4, 1))).astype(np.float16),
        "b3rep": np.ascontiguousarray(np.tile(
            b3.reshape(COUT, CIN).T.reshape(-1), (128, 1))),
        "xT": np.ascontiguousarray(x.T).astype(np.float16),
    }
    jp = np.arange(512)
    in_maps = []
    for c in range(NCORES):
        m_of = 4095 - 16 * c - 128 * (jp // 16) - (jp % 16)
        tq = t_new[m_of].reshape(4, 128)
        t1 = np.repeat(tq[:, None, :], 32, axis=1).reshape(128, 128)
        im = dict(common)
        im["t1rep"] = np.ascontiguousarray(t1)
        in_maps.append(im)
    return in_maps


def _host_combine(outs):
    Z = np.zeros((N + 256, COUT), dtype=np.float64)
    for c in range(NCORES):
        oc = np.asarray(outs[c]).reshape(8, 4, 32, 512)  # [T, g, o, dn]
        for g in range(4):
            seq = oc[:, g].transpose(0, 2, 1).reshape(N, COUT)
            s = 16 * c + 4 * g
            Z[s:s + N] += seq
    return Z[:N].astype(f32)


def kernel(**inputs):
    from concourse import bass_utils

    t = np.asarray(inputs["t"])
    t_min = int(t.min())
    idx = t - t_min
    assert int(t.max()) - t_min + 1 == N, "kernel hardcodes N=4096"

    # scatter observations onto the regular grid (identity when t is arange)
    x_in = np.asarray(inputs["x"], dtype=f32)
    x_new = np.zeros((N, CIN), dtype=f32)
    x_new[idx] = x_in
    ins = dict(inputs)
    ins["x"] = x_new

    if "prog" not in _CACHE:
        _CACHE["prog"] = _build_program()
    nc = _CACHE["prog"]

    in_maps = _host_prep(ins)
    res = bass_utils.run_bass_kernel_spmd(
        nc, in_maps, core_ids=list(range(NCORES)))
    outs = [res.results[c]["out"] for c in range(NCORES)]
    z = _host_combine(outs)
    return z[idx]


if __name__ == "__main__":
    import jax
    cpu = jax.devices("cpu")[0]
    with jax.default_device(cpu):
        sys.path.insert(0, os.path.dirname(os.path.abspath(__file__)))
        import reference as R
        inputs = {k: np.asarray(v) for k, v in R.setup_inputs().items()}
        import jax.numpy as jnp
        z0 = np.asarray(R.reference(**{k: jnp.asarray(v)
                                       for k, v in inputs.items()}))
    z = kernel(**inputs)
    rel = np.linalg.norm(z - z0) / np.linalg.norm(z0)
    print("Relative error:", rel)


# revision 10
# speedup vs baseline: 2.9203x; 1.0727x over previous
"""Trainium2 Bass kernel for nn_CKConv (SIREN kernel-net + causal conv1d).

Decomposition (8 cores, SPMD — identical program, per-core data):
  z[n,o] = sum_{ci, l<=n} W[o,ci,4095-l] * x[n-l,ci],  W[o,ci,m]=weights[m,32o+ci]
Tap l = 128u + 16c + 4g + dl  (c = core, u in [0,32), g,dl in [0,4)).
Each core computes SIREN weights for its 512 taps (j' = 16u+4g+dl, packed
[128,128]), writes them to DRAM, gathers them back as conv lhsT tiles
[(dl,ci),(g,o)], and runs 144 accumulating [K=128,M=128,N=512] matmuls against
a 4-shift x image XS.  psum row (g,o) of output tile T holds the contribution
to z[512T+dn+4g+16c, o]; the host sums the shifted partials.

Numerics: layer-1 sin args (up to ~3.3e5 rad) are computed with the reference's
exact fp32 rounding sequence and evaluated with a Cody-Waite range reduction
(6x7-bit pieces of 2pi; k*Pi products exact) + degree-11 odd polynomial, giving
|sin err| < 1e-6 and final rel err ~1e-5 vs the fp32 jax reference.
"""
import os
import sys
import numpy as np

if "/opt/trn_rl_repo" not in sys.path:
    sys.path.insert(0, "/opt/trn_rl_repo")

f32 = np.float32
OMEGA = 32.5
N, CIN, COUT, H = 4096, 32, 32, 32
NCORES = 8
PAD = 512
XSW = PAD + N + 4

INV2PI = 0.15915493667125702
MAGIC = 12582912.0  # 1.5 * 2^23
P6 = [6.3125, -0.029296875, -1.7881393432617188e-05, 6.332993507385254e-08,
      2.4374458007514477e-10, -6.608047442568932e-13]
Q2 = [6.2831878662109375, -2.559034328442067e-06]
CPOLY = [0.9999995827674866, -0.1666654497385025, 0.008332346566021442,
         -0.00019806942145805806, 2.697602667467436e-06, -2.0269567357900087e-08]

_CACHE = {}


def _build_program():
    import concourse.bacc as bacc
    import concourse.mybir as mybir
    import concourse.tile as tile

    dt = mybir.dt.float32
    AF = mybir.ActivationFunctionType
    OP = mybir.AluOpType

    nc = bacc.Bacc("TRN2", target_bir_lowering=False, debug=False,
                   num_devices=NCORES)

    # ---------------- DRAM I/O ----------------
    d_cst = nc.dram_tensor("consts", [128, 196], dt, kind="ExternalInput")
    d_w3 = nc.dram_tensor("w3rep", [128, 1024], mybir.dt.float16,
                          kind="ExternalInput")
    d_b3 = nc.dram_tensor("b3rep", [128, 1024], dt, kind="ExternalInput")
    d_xT = nc.dram_tensor("xT", [32, 4096], mybir.dt.float16,
                          kind="ExternalInput")
    d_out = nc.dram_tensor("out", [8, 128, 512], dt, kind="ExternalOutput")

    with tile.TileContext(nc) as tc:
        with (
            tc.tile_pool(name="const", bufs=1) as cp,
            tc.tile_pool(name="sin", bufs=1) as sp,
            tc.tile_pool(name="wsb", bufs=2) as wp,
            tc.tile_pool(name="lhs", bufs=1) as lp,
            tc.tile_pool(name="osb", bufs=2) as op_,
            tc.tile_pool(name="ps2", bufs=1, space="PSUM") as pp2,
            tc.tile_pool(name="ps3", bufs=1, space="PSUM") as pp3,
            tc.tile_pool(name="psc", bufs=1, space="PSUM") as ppc,
            tc.tile_pool(name="wdram", bufs=1, space="DRAM") as dp,
        ):
            # ---------------- load constants ----------------
            def load(dram, shape, tag):
                t = cp.tile(shape, dt, tag=tag)
                nc.sync.dma_start(t[:], dram[:])
                return t

            cst = load(d_cst, [128, 196], "cst")
            t1 = cst[:, 0:128]
            w1r = cst[:, 128:129]
            b1r = cst[:, 129:130]
            og2 = cst[:, 130:131]
            ob2 = cst[:, 131:132]
            v2r = cst[:, 132:164]
            v2Tr = cst[:, 164:196]
            w3r = cp.tile([128, 1024], mybir.dt.float16, tag="w3r")
            nc.sync.dma_start(w3r[:], d_w3[:])
            b3r = load(d_b3, [128, 1024], "b3r")

            # XS: [128=(4ci+dl), XSW];  XS[(ci,dl), PAD+dl+j] = x[j,ci]
            xs = cp.tile([128, XSW], mybir.dt.float16, tag="xs")
            nc.vector.memset(xs[:, 0:516], 0.0)
            xs_r = xs[:].rearrange("(ci dl) w -> dl ci w", dl=4)
            for dl in range(4):
                nc.sync.dma_start(
                    xs_r[dl, :, PAD + dl:PAD + dl + 4096], d_xT[:])

            # ---------------- sin helper ----------------
            def dev_sin(out, arg, pieces, nm):
                v = sp.tile([128, 128], dt, tag=f"{nm}v")
                k = sp.tile([128, 128], dt, tag=f"{nm}k")
                t_ = sp.tile([128, 128], dt, tag=f"{nm}t")
                r = sp.tile([128, 128], dt, tag=f"{nm}r")
                u = sp.tile([128, 128], dt, tag=f"{nm}u")
                a = sp.tile([128, 128], dt, tag=f"{nm}a")
                nc.vector.tensor_scalar(v[:], arg, INV2PI, MAGIC,
                                        op0=OP.mult, op1=OP.add)
                nc.vector.tensor_scalar_sub(k[:], v[:], MAGIC)
                # r_i' = (k*s_i) - r_{i-1}' with s_i = (-1)^(i+1) P_i keeps
                # each step one fused op; even piece-count ends with r' = r,
                # and fl(t-r) = -fl(r-t) under RN so bits are unchanged.
                cur, sgn = arg, 1.0
                for i, p in enumerate(pieces):
                    dst = r if (i % 2 == 0) else t_
                    nc.vector.scalar_tensor_tensor(
                        dst[:], k[:], sgn * float(p), cur,
                        op0=OP.mult, op1=OP.subtract)
                    cur, sgn = dst[:], -sgn
                nc.vector.tensor_mul(u[:], cur, cur)
                nc.vector.tensor_scalar(a[:], u[:], CPOLY[5], CPOLY[4],
                                        op0=OP.mult, op1=OP.add)
                for ci in (3, 2, 1, 0):
                    nc.vector.tensor_mul(a[:], a[:], u[:])
                    nc.vector.tensor_scalar_add(a[:], a[:], CPOLY[ci])
                nc.vector.tensor_mul(out, a[:], cur)

            # ---------------- layer 1 ----------------
            # arg1 = fl(fl(fl(t*w1)+b1)*OMEGA)  (reference's exact roundings)
            a1 = sp.tile([128, 128], dt, tag="a1")
            nc.vector.tensor_scalar(a1[:], t1, w1r, b1r,
                                    op0=OP.mult, op1=OP.add)
            nc.vector.tensor_scalar_mul(a1[:], a1[:], OMEGA)
            h1q = cp.tile([128, 128], dt, tag="h1q")
            dev_sin(h1q[:], a1[:], P6, "s1")

            # ---------------- layer 2 ----------------
            # psum2q[(q,h'),mm] = sum_h v2[h',h] * h1[h,128q+mm]
            ps2 = pp2.tile([128, 128], dt)
            for q in range(4):
                nc.tensor.matmul(ps2[32 * q:32 * q + 32, :],
                                 v2Tr[32 * q:32 * q + 32],
                                 h1q[32 * q:32 * q + 32, :],
                                 start=True, stop=True,
                                 tile_position=(32 * q, 32 * q))
            # scaleA = omg2 * 1/sqrt(sum v2^2)
            sq = sp.tile([128, 32], dt, tag="sq")
            n2 = sp.tile([128, 1], dt, tag="n2")
            sca = sp.tile([128, 1], dt, tag="sca")
            nc.vector.tensor_mul(sq[:], v2r, v2r)
            nc.vector.tensor_reduce(n2[:], sq[:], mybir.AxisListType.X, OP.add)
            nc.scalar.activation(n2[:], n2[:], AF.Sqrt)
            nc.vector.reciprocal(n2[:], n2[:])
            nc.vector.tensor_mul(sca[:], og2, n2[:])
            a2 = sp.tile([128, 128], dt, tag="a2")
            nc.vector.tensor_scalar(a2[:], ps2[:], sca[:, 0:1], ob2,
                                    op0=OP.mult, op1=OP.add)
            h2q = cp.tile([128, 128], mybir.dt.float16, tag="h2q")
            dev_sin(h2q[:], a2[:], Q2, "s2")

            # ---------------- layer 3 + gather + conv ----------------
            # Wnat[128Jb+mm, f] = sum_h h2q[32Jb+h, mm] * w3rep[32Jb+h, f] + b3
            lhsTb = []
            for Jb in range(4):
                ps3 = pp3.tile([128, 1024], dt)
                for fb in range(2):
                    nc.tensor.matmul(ps3[:, 512 * fb:512 * fb + 512],
                                     h2q[32 * Jb:32 * Jb + 32, :],
                                     w3r[32 * Jb:32 * Jb + 32,
                                         512 * fb:512 * fb + 512],
                                     start=True, stop=True,
                                     tile_position=(32 * Jb, 0))
                wsb = wp.tile([128, 1024], mybir.dt.float16, tag="wsb")
                nc.vector.tensor_add(wsb[:], ps3[:], b3r[:])
                wd = dp.tile([128, 1024], mybir.dt.float16, tag=f"wd{Jb}")
                nc.sync.dma_start(wd[:], wsb[:])
                # gather: lhsTb[Jb][(4ci+dl), 128us+32g+o] = Wnat[16us+4g+dl, 32ci+o]
                lb = lp.tile([128, 1024], mybir.dt.float16, tag=f"lb{Jb}")
                src = wd[:].rearrange("(usg dl) (ci o) -> dl ci usg o",
                                      dl=4, o=32)
                dst = lb[:].rearrange("(ci dl) f -> dl ci f", dl=4)
                for dl in range(4):
                    nc.sync.dma_start(dst[dl], src[dl])
                lhsTb.append(lb)

            # conv: u-outer over T-halves so each stationary lhsT_u serves
            # up to 4 output tiles per LDWEIGHTS (PE.SEQ is the bottleneck)
            for half in (range(0, 4), range(4, 8)):
                pscs = {T: ppc.tile([128, 512], dt, tag=f"c{T % 4}",
                                    name=f"psc{T}")
                        for T in half}
                numax = 4 * (half[-1] + 1)
                for u in range(numax):
                    Jb, us = u // 8, u % 8
                    lt = lhsTb[Jb][:, 128 * us:128 * us + 128]
                    for T in half:
                        nu = 4 * (T + 1)
                        if u >= nu:
                            continue
                        off = PAD + 512 * T - 128 * u
                        nc.tensor.matmul(pscs[T][:], lt,
                                         xs[:, off:off + 512],
                                         start=(u == 0), stop=(u == nu - 1))
                        if u == nu - 1:
                            osb = op_.tile([128, 512], dt, tag="osb")
                            nc.vector.tensor_copy(osb[:], pscs[T][:])
                            nc.sync.dma_start(d_out[T], osb[:])

    nc.finalize()
    return nc


def _host_prep(inputs):
    """Per-core input maps.  Only t1rep differs across cores."""
    import jax
    import jax.numpy as jnp
    cpu = jax.devices("cpu")[0]
    with jax.default_device(cpu):
        t_new = np.asarray(jnp.linspace(-1.0, 2.0 * (N / 1.0) - 1.0, N))
    t_new = t_new.astype(f32)

    x = np.asarray(inputs["x"], dtype=f32)
    v1 = np.asarray(inputs["v1"], dtype=f32)
    g1 = np.asarray(inputs["g1"], dtype=f32)
    b1 = np.asarray(inputs["b1"], dtype=f32)
    v2 = np.asarray(inputs["v2"], dtype=f32)
    g2 = np.asarray(inputs["g2"], dtype=f32)
    b2 = np.asarray(inputs["b2"], dtype=f32)
    w3 = np.asarray(inputs["w3"], dtype=f32)
    b3 = np.asarray(inputs["b3"], dtype=f32)

    w1 = (g1[:, None] * v1 / np.linalg.norm(v1, axis=1, keepdims=True)
          ).astype(f32)[:, 0]

    def packed_consts(t1):
        return np.ascontiguousarray(np.concatenate([
            t1,
            np.tile(w1, 4)[:, None], np.tile(b1, 4)[:, None],
            f32(OMEGA * np.tile(g2, 4))[:, None],
            f32(OMEGA * np.tile(b2, 4))[:, None],
            np.tile(v2, (4, 1)), np.tile(v2.T, (4, 1)),
        ], axis=1, dtype=f32))

    common = {
        "w3rep": np.ascontiguousarray(np.tile(
            np.transpose(w3.reshape(COUT, CIN, H), (2, 1, 0)
                         ).reshape(H, CIN * COUT), (4, 1))).astype(np.float16),
        "b3rep": np.ascontiguousarray(np.tile(
            b3.reshape(COUT, CIN).T.reshape(-1), (128, 1))),
        "xT": np.ascontiguousarray(x.T).astype(np.float16),
    }
    jp = np.arange(512)
    in_maps = []
    for c in range(NCORES):
        m_of = 4095 - 16 * c - 128 * (jp // 16) - (jp % 16)
        tq = t_new[m_of].reshape(4, 128)
        t1 = np.repeat(tq[:, None, :], 32, axis=1).reshape(128, 128)
        im = dict(common)
        im["consts"] = packed_consts(t1)
        in_maps.append(im)
    return in_maps


def _host_combine(outs):
    Z = np.zeros((N + 256, COUT), dtype=np.float64)
    for c in range(NCORES):
        oc = np.asarray(outs[c]).reshape(8, 4, 32, 512)  # [T, g, o, dn]
        for g in range(4):
            seq = oc[:, g].transpose(0, 2, 1).reshape(N, COUT)
            s = 16 * c + 4 * g
            Z[s:s + N] += seq
    return Z[:N].astype(f32)


def kernel(**inputs):
    from concourse import bass_utils

    t = np.asarray(inputs["t"])
    t_min = int(t.min())
    idx = t - t_min
    assert int(t.max()) - t_min + 1 == N, "kernel hardcodes N=4096"

    # scatter observations onto the regular grid (identity when t is arange)
    x_in = np.asarray(inputs["x"], dtype=f32)
    x_new = np.zeros((N, CIN), dtype=f32)
    x_new[idx] = x_in
    ins = dict(inputs)
    ins["x"] = x_new

    if "prog" not in _CACHE:
        _CACHE["prog"] = _build_program()
    nc = _CACHE["prog"]

    in_maps = _host_prep(ins)
    res = bass_utils.run_bass_kernel_spmd(
        nc, in_maps, core_ids=list(range(NCORES)))
    outs = [res.results[c]["out"] for c in range(NCORES)]
    z = _host_combine(outs)
    return z[idx]


if __name__ == "__main__":
    import jax
    cpu = jax.devices("cpu")[0]
    with jax.default_device(cpu):
        sys.path.insert(0, os.path.dirname(os.path.abspath(__file__)))
        import reference as R
        inputs = {k: np.asarray(v) for k, v in R.setup_inputs().items()}
        import jax.numpy as jnp
        z0 = np.asarray(R.reference(**{k: jnp.asarray(v)
                                       for k, v in inputs.items()}))
    z = kernel(**inputs)
    rel = np.linalg.norm(z - z0) / np.linalg.norm(z0)
    print("Relative error:", rel)
